# revision 13
# baseline (speedup 1.0000x reference)
"""GNN message-passing ConvNet layer on 8 TRN2 NeuronCores (Bass/Tile).

Computes, for x [B=4, N=4096, D=128], adj_mat [B, N, N] (0/1 floats),
U [D, D]:
    deg[b, i] = sum_j adj_mat[b, j, i]
    agg[b, i, :] = sum_j adj[b, j, i] * x[b, j, :]
    out = relu((agg @ U) / deg[..., None])

Sharding: core c handles batch c//2 and destination-node half c%2 — no
collectives; each core reads its own adjacency column slice once.

V2 kernel (per core, memory-bound):
  - Associativity: (A^T x) U == A^T (x U). Precompute y = x @ U once on
    the PE, quantize to bf16; the adjacency pass then emits the final
    pre-relu output directly (no U-matmul tail).
  - adj is 0/1: cast to bf16 on host (exact), host-packed to the SBUF
    tile order [p, u, q, n] so DMAs are 16 KiB/partition contiguous.
  - j-outer loop: stationary y_t loaded once per tile serves all 4
    i-round matmuls (amortizes LDWEIGHTS, which profiling showed was
    serialized at ~45ns/MM), accumulating into 4 PSUM banks; deg via
    ones-stationary matmuls into one shared PSUM bank at partitions
    0/32/64/96.
  - Final chunk runs deg matmuls first so the reciprocal+broadcast of
    1/deg overlaps the last agg matmuls.
"""

import os
import sys

for _p in ("/opt/trn_rl_repo",):
    if _p not in sys.path and os.path.isdir(_p):
        sys.path.insert(0, _p)

from contextlib import ExitStack

import numpy as np

B, N, D = 4, 4096, 128
P = 128
N_CORES = 8

_PROG = None


def _build_program(n=N, i_core=N // 2, d=D, w=512, jt_per_dma=8):
    from concourse import mybir, tile, bacc

    f32 = mybir.dt.float32
    f32r = mybir.dt.float32r
    bf16 = mybir.dt.bfloat16
    n_jt = n // P              # 32 j-tiles of 128
    n_rounds = i_core // w     # 4 i-rounds of 512
    n_chunks = n_jt // jt_per_dma
    xt_chunk = 8               # j-tiles per xT DMA (spread across rings)

    nc = bacc.Bacc(
        "TRN2",
        target_bir_lowering=False,
        debug=False,
        enable_asserts=True,
        num_devices=N_CORES,
    )
    # host-packed: adj_p[p, u, q, n] = adj[b, u*128+p, i0 + q*512 + n]
    adj_d = nc.dram_tensor("adj_p", [P, n_jt, n_rounds, w], bf16, kind="ExternalInput")
    # host-packed transpose: xT_p[d, t, j] = x[b, t*128+j, d]
    xt_d = nc.dram_tensor("xT_p", [P, n_jt, d], f32r, kind="ExternalInput")
    u_d = nc.dram_tensor("U", [d, d], f32r, kind="ExternalInput")
    ones_d = nc.dram_tensor("ones_c", [P, 1], bf16, kind="ExternalInput")
    # out_sp[q, e, n] = out[b, i0 + q*512 + n, e]  (host transposes back)
    out_d = nc.dram_tensor("out_sp", [n_rounds, d, w], f32, kind="ExternalOutput")

    with tile.TileContext(nc, trace_sim=False) as tc, ExitStack() as ctx:
        const_pool = ctx.enter_context(tc.tile_pool(name="const", bufs=1))
        y_pool = ctx.enter_context(tc.tile_pool(name="y", bufs=1))
        adj_pool = ctx.enter_context(tc.tile_pool(name="adj", bufs=3))
        scale_pool = ctx.enter_context(tc.tile_pool(name="scale", bufs=4))
        out_pool = ctx.enter_context(tc.tile_pool(name="out", bufs=8))
        small_pool = ctx.enter_context(tc.tile_pool(name="small", bufs=4))
        ps_agg = ctx.enter_context(tc.tile_pool(name="ps_agg", bufs=1, space="PSUM"))
        # 4 banks shared in time: y-precompute tiles first, then one deg
        # accumulator per i-round (separate banks - PSUM accumulation-group
        # zero regions are bank-wide).
        ps_aux = ctx.enter_context(tc.tile_pool(name="ps_aux", bufs=1, space="PSUM"))

        xt_sb = const_pool.tile([P, n_jt, d], f32r)
        for xc in range(n_jt // xt_chunk):
            nc.scalar.dma_start(
                xt_sb[:, xc * xt_chunk : (xc + 1) * xt_chunk, :],
                xt_d[:, xc * xt_chunk : (xc + 1) * xt_chunk, :],
            )
        u_sb = const_pool.tile([P, d], f32r)
        nc.scalar.dma_start(u_sb[:], u_d[:])
        ones = const_pool.tile([P, 1], bf16)
        nc.scalar.dma_start(ones[:], ones_d[:])

        # Phase 0: y = x @ U, quantized to bf16, laid out [j_in_tile, t, e].
        y_sb = y_pool.tile([P, n_jt, d], bf16)
        for t in range(n_jt):
            y_ps = ps_aux.tile([P, d], f32, tag=f"d{t % 4}", name=f"y{t}")
            nc.tensor.matmul(y_ps[:], xt_sb[:, t, :], u_sb[:], start=True, stop=True)
            nc.vector.tensor_copy(y_sb[:, t, :], y_ps[:])

        # Phase 1: stream adjacency once, j-outer. agg round q accumulates
        # into its own PSUM bank; all deg rounds share one bank at
        # partitions 0/32/64/96.
        agg_ps = [ps_agg.tile([P, w], f32, tag=f"agg{q}", name=f"agg{q}")
                  for q in range(n_rounds)]
        deg_ps = [ps_aux.tile([1, w], f32, tag=f"d{q}", name=f"deg{q}")
                  for q in range(n_rounds)]
        for c in range(n_chunks):
            adj_sb = adj_pool.tile([P, jt_per_dma, n_rounds, w], bf16, tag="adj")
            nc.sync.dma_start(
                adj_sb[:], adj_d[:, c * jt_per_dma : (c + 1) * jt_per_dma, :, :]
            )
            first, last = c == 0, c == n_chunks - 1

            def agg_mms():
                for u in range(jt_per_dma):
                    t = c * jt_per_dma + u
                    for q in range(n_rounds):
                        nc.tensor.matmul(
                            agg_ps[q][:],
                            y_sb[:, t, :],
                            adj_sb[:, u, q, :],
                            start=(first and u == 0),
                            stop=(last and u == jt_per_dma - 1),
                        )

            def deg_mms():
                for u in range(jt_per_dma):
                    for q in range(n_rounds):
                        nc.tensor.matmul(
                            deg_ps[q][:],
                            ones[:],
                            adj_sb[:, u, q, :],
                            start=(first and u == 0),
                            stop=(last and u == jt_per_dma - 1),
                        )

            # Last chunk: deg first so the 1/deg tail overlaps final aggs.
            if last:
                deg_mms()
                agg_mms()
            else:
                agg_mms()
                deg_mms()

        for q in range(n_rounds):
            recip = small_pool.tile([1, w], f32, tag="recip")
            nc.vector.reciprocal_approx_fast(recip[:], deg_ps[q][:])
            rb = scale_pool.tile([P, w], f32, tag="rb")
            nc.gpsimd.partition_broadcast(rb[:], recip[:])
            relu_sb = out_pool.tile([P, w], f32, tag="relu")
            nc.scalar.activation(
                relu_sb[:], agg_ps[q][:], mybir.ActivationFunctionType.Relu
            )
            out_sb = out_pool.tile([P, w], f32, tag="osb")
            nc.vector.tensor_mul(out_sb[:], relu_sb[:], rb[:])
            (nc.scalar if q % 2 else nc.sync).dma_start(out_d[q, :, :], out_sb[:])

    nc.compile()
    return nc


def _get_program():
    global _PROG
    if _PROG is None:
        _PROG = _build_program()
    return _PROG


def _shard_inputs(x, adj_mat, U):
    import ml_dtypes

    bf16 = ml_dtypes.bfloat16
    i_core = N // 2
    n_jt = N // P
    n_rounds = i_core // 512
    ones_c = np.ones((P, 1), dtype=bf16)
    u_f = np.ascontiguousarray(U, dtype=np.float32)
    adj_bf = adj_mat.astype(bf16)  # exact: values are 0/1
    in_maps = []
    for c in range(N_CORES):
        b, half = c // 2, c % 2
        i0 = half * i_core
        # [N, i_core] -> [u, p, q, n] -> [p, u, q, n]
        a = adj_bf[b, :, i0 : i0 + i_core].reshape(n_jt, P, n_rounds, 512)
        a = np.ascontiguousarray(a.transpose(1, 0, 2, 3))
        xt = np.ascontiguousarray(
            x[b].reshape(n_jt, P, D).transpose(2, 0, 1), dtype=np.float32
        )
        in_maps.append(
            {"adj_p": a, "xT_p": xt, "U": u_f, "ones_c": ones_c}
        )
    return in_maps


def _run(x, adj_mat, U, trace=False):
    from concourse.bass_utils import run_bass_kernel_spmd

    nc = _get_program()
    in_maps = _shard_inputs(x, adj_mat, U)
    res = run_bass_kernel_spmd(
        nc, in_maps, core_ids=list(range(N_CORES)), trace=trace
    )
    i_core = N // 2
    out = np.empty((B, N, D), dtype=np.float32)
    for c in range(N_CORES):
        b, half = c // 2, c % 2
        i0 = half * i_core
        osp = res.results[c]["out_sp"]  # [q, e, n]
        out[b, i0 : i0 + i_core, :] = osp.transpose(0, 2, 1).reshape(i_core, D)
    return out, res


def kernel(x, adj_mat, U):
    out, _ = _run(
        np.asarray(x, dtype=np.float32),
        np.asarray(adj_mat, dtype=np.float32),
        np.asarray(U, dtype=np.float32),
    )
    return out


# revision 14
# speedup vs baseline: 1.3589x; 1.3589x over previous
"""GNN message-passing ConvNet layer on 8 TRN2 NeuronCores (Bass/Tile).

Computes, for x [B=4, N=4096, D=128], adj_mat [B, N, N] (0/1 floats),
U [D, D]:
    deg[b, i] = sum_j adj_mat[b, j, i]
    agg[b, i, :] = sum_j adj[b, j, i] * x[b, j, :]
    out = relu((agg @ U) / deg[..., None])

Sharding: core c handles batch c//2 and destination-node half c%2 — no
collectives; each core reads its own adjacency column slice once.

V2 kernel (per core, memory-bound):
  - Associativity: (A^T x) U == A^T (x U). Precompute y = x @ U once on
    the PE, quantize to bf16; the adjacency pass then emits the final
    pre-relu output directly (no U-matmul tail).
  - adj is 0/1: cast to bf16 on host (exact), host-packed to the SBUF
    tile order [p, u, q, n] so DMAs are 16 KiB/partition contiguous.
  - j-outer loop: stationary y_t loaded once per tile serves all 4
    i-round matmuls (amortizes LDWEIGHTS, which profiling showed was
    serialized at ~45ns/MM), accumulating into 4 PSUM banks; deg via
    ones-stationary matmuls into one shared PSUM bank at partitions
    0/32/64/96.
  - Final chunk runs deg matmuls first so the reciprocal+broadcast of
    1/deg overlaps the last agg matmuls.
"""

import os
import sys

for _p in ("/opt/trn_rl_repo",):
    if _p not in sys.path and os.path.isdir(_p):
        sys.path.insert(0, _p)

from contextlib import ExitStack

import numpy as np

B, N, D = 4, 4096, 128
P = 128
N_CORES = 8

_PROG = None


def _build_program(n=N, i_core=N // 2, d=D, w=512, jt_per_dma=4):
    from concourse import mybir, tile, bacc

    f32 = mybir.dt.float32
    f32r = mybir.dt.float32r
    bf16 = mybir.dt.bfloat16
    n_jt = n // P              # 32 j-tiles of 128
    n_rounds = i_core // w     # 4 i-rounds of 512
    n_chunks = n_jt // jt_per_dma
    xt_chunk = 8               # j-tiles per xT DMA (spread across rings)

    nc = bacc.Bacc(
        "TRN2",
        target_bir_lowering=False,
        debug=False,
        enable_asserts=True,
        num_devices=N_CORES,
    )
    # host-packed: adj_p[p, u, q, n] = adj[b, u*128+p, i0 + q*512 + n]
    adj_d = nc.dram_tensor("adj_p", [P, n_jt, n_rounds, w], bf16, kind="ExternalInput")
    # host-packed transpose: xT_p[d, t, j] = x[b, t*128+j, d]
    xt_d = nc.dram_tensor("xT_p", [P, n_jt, d], f32r, kind="ExternalInput")
    u_d = nc.dram_tensor("U", [d, d], f32r, kind="ExternalInput")
    ones_d = nc.dram_tensor("ones_c", [P, 1], bf16, kind="ExternalInput")
    # out_sp[q, e, n] = out[b, i0 + q*512 + n, e]  (host transposes back)
    out_d = nc.dram_tensor("out_sp", [n_rounds, d, w], f32, kind="ExternalOutput")

    with tile.TileContext(nc, trace_sim=False) as tc, ExitStack() as ctx:
        const_pool = ctx.enter_context(tc.tile_pool(name="const", bufs=1))
        y_pool = ctx.enter_context(tc.tile_pool(name="y", bufs=1))
        adj_pool = ctx.enter_context(tc.tile_pool(name="adj", bufs=4))
        scale_pool = ctx.enter_context(tc.tile_pool(name="scale", bufs=4))
        out_pool = ctx.enter_context(tc.tile_pool(name="out", bufs=8))
        small_pool = ctx.enter_context(tc.tile_pool(name="small", bufs=4))
        ps_agg = ctx.enter_context(tc.tile_pool(name="ps_agg", bufs=1, space="PSUM"))
        # 4 banks shared in time: y-precompute tiles first, then one deg
        # accumulator per i-round (separate banks - PSUM accumulation-group
        # zero regions are bank-wide).
        ps_aux = ctx.enter_context(tc.tile_pool(name="ps_aux", bufs=1, space="PSUM"))

        u_sb = const_pool.tile([P, d], f32r)
        nc.sync.dma_start(u_sb[:], u_d[:])
        ones = const_pool.tile([P, 1], bf16)
        nc.sync.dma_start(ones[:], ones_d[:])
        xt_sb = const_pool.tile([P, n_jt, d], f32r)
        for xc in range(n_jt // xt_chunk):
            nc.sync.dma_start(
                xt_sb[:, xc * xt_chunk : (xc + 1) * xt_chunk, :],
                xt_d[:, xc * xt_chunk : (xc + 1) * xt_chunk, :],
            )

        # Phase 0: y = x @ U, quantized to bf16, laid out [j_in_tile, t, e].
        y_sb = y_pool.tile([P, n_jt, d], bf16)
        for t in range(n_jt):
            y_ps = ps_aux.tile([P, d], f32, tag=f"d{t % 4}", name=f"y{t}")
            nc.tensor.matmul(y_ps[:], xt_sb[:, t, :], u_sb[:], start=True, stop=True)
            nc.vector.tensor_copy(y_sb[:, t, :], y_ps[:])

        # Phase 1: stream adjacency once, j-outer. agg round q accumulates
        # into its own PSUM bank; all deg rounds share one bank at
        # partitions 0/32/64/96.
        agg_ps = [ps_agg.tile([P, w], f32, tag=f"agg{q}", name=f"agg{q}")
                  for q in range(n_rounds)]
        deg_ps = [ps_aux.tile([1, w], f32, tag=f"d{q}", name=f"deg{q}")
                  for q in range(n_rounds)]
        for c in range(n_chunks):
            adj_sb = adj_pool.tile([P, jt_per_dma, n_rounds, w], bf16, tag="adj")
            nc.sync.dma_start(
                adj_sb[:], adj_d[:, c * jt_per_dma : (c + 1) * jt_per_dma, :, :]
            )
            first, last = c == 0, c == n_chunks - 1

            def agg_mms():
                for u in range(jt_per_dma):
                    t = c * jt_per_dma + u
                    for q in range(n_rounds):
                        nc.tensor.matmul(
                            agg_ps[q][:],
                            y_sb[:, t, :],
                            adj_sb[:, u, q, :],
                            start=(first and u == 0),
                            stop=(last and u == jt_per_dma - 1),
                        )

            def deg_mms():
                for u in range(jt_per_dma):
                    for q in range(n_rounds):
                        nc.tensor.matmul(
                            deg_ps[q][:],
                            ones[:],
                            adj_sb[:, u, q, :],
                            start=(first and u == 0),
                            stop=(last and u == jt_per_dma - 1),
                        )

            # Last chunk: deg first so the 1/deg tail overlaps final aggs.
            if last:
                deg_mms()
                agg_mms()
            else:
                agg_mms()
                deg_mms()

        for q in range(n_rounds):
            recip = small_pool.tile([1, w], f32, tag="recip")
            nc.vector.reciprocal_approx_fast(recip[:], deg_ps[q][:])
            rb = scale_pool.tile([P, w], f32, tag="rb")
            nc.gpsimd.partition_broadcast(rb[:], recip[:])
            relu_sb = out_pool.tile([P, w], f32, tag="relu")
            nc.scalar.activation(
                relu_sb[:], agg_ps[q][:], mybir.ActivationFunctionType.Relu
            )
            out_sb = out_pool.tile([P, w], f32, tag="osb")
            nc.vector.tensor_mul(out_sb[:], relu_sb[:], rb[:])
            (nc.scalar if q % 2 else nc.gpsimd).dma_start(out_d[q, :, :], out_sb[:])

    nc.compile()
    return nc


def _get_program():
    global _PROG
    if _PROG is None:
        _PROG = _build_program()
    return _PROG


def _shard_inputs(x, adj_mat, U):
    import ml_dtypes

    bf16 = ml_dtypes.bfloat16
    i_core = N // 2
    n_jt = N // P
    n_rounds = i_core // 512
    ones_c = np.ones((P, 1), dtype=bf16)
    u_f = np.ascontiguousarray(U, dtype=np.float32)
    adj_bf = adj_mat.astype(bf16)  # exact: values are 0/1
    in_maps = []
    for c in range(N_CORES):
        b, half = c // 2, c % 2
        i0 = half * i_core
        # [N, i_core] -> [u, p, q, n] -> [p, u, q, n]
        a = adj_bf[b, :, i0 : i0 + i_core].reshape(n_jt, P, n_rounds, 512)
        a = np.ascontiguousarray(a.transpose(1, 0, 2, 3))
        xt = np.ascontiguousarray(
            x[b].reshape(n_jt, P, D).transpose(2, 0, 1), dtype=np.float32
        )
        in_maps.append(
            {"adj_p": a, "xT_p": xt, "U": u_f, "ones_c": ones_c}
        )
    return in_maps


def _run(x, adj_mat, U, trace=False):
    from concourse.bass_utils import run_bass_kernel_spmd

    nc = _get_program()
    in_maps = _shard_inputs(x, adj_mat, U)
    res = run_bass_kernel_spmd(
        nc, in_maps, core_ids=list(range(N_CORES)), trace=trace
    )
    i_core = N // 2
    out = np.empty((B, N, D), dtype=np.float32)
    for c in range(N_CORES):
        b, half = c // 2, c % 2
        i0 = half * i_core
        osp = res.results[c]["out_sp"]  # [q, e, n]
        out[b, i0 : i0 + i_core, :] = osp.transpose(0, 2, 1).reshape(i_core, D)
    return out, res


def kernel(x, adj_mat, U):
    out, _ = _run(
        np.asarray(x, dtype=np.float32),
        np.asarray(adj_mat, dtype=np.float32),
        np.asarray(U, dtype=np.float32),
    )
    return out


# revision 15
# speedup vs baseline: 1.4216x; 1.0462x over previous
"""GNN message-passing ConvNet layer on 8 TRN2 NeuronCores (Bass/Tile).

Computes, for x [B=4, N=4096, D=128], adj_mat [B, N, N] (0/1 floats),
U [D, D]:
    deg[b, i] = sum_j adj_mat[b, j, i]
    agg[b, i, :] = sum_j adj[b, j, i] * x[b, j, :]
    out = relu((agg @ U) / deg[..., None])

Sharding: core c handles batch c//2 and destination-node half c%2 — no
collectives; each core reads its own adjacency column slice once.

V2 kernel (per core, memory-bound):
  - Associativity: (A^T x) U == A^T (x U). Precompute y = x @ U once on
    the PE, quantize to bf16; the adjacency pass then emits the final
    pre-relu output directly (no U-matmul tail).
  - adj is 0/1: cast to bf16 on host (exact), host-packed to the SBUF
    tile order [p, u, q, n] so DMAs are 16 KiB/partition contiguous.
  - j-outer loop: stationary y_t loaded once per tile serves all 4
    i-round matmuls (amortizes LDWEIGHTS, which profiling showed was
    serialized at ~45ns/MM), accumulating into 4 PSUM banks; deg via
    ones-stationary matmuls into one shared PSUM bank at partitions
    0/32/64/96.
  - Final chunk runs deg matmuls first so the reciprocal+broadcast of
    1/deg overlaps the last agg matmuls.
"""

import os
import sys

for _p in ("/opt/trn_rl_repo",):
    if _p not in sys.path and os.path.isdir(_p):
        sys.path.insert(0, _p)

from contextlib import ExitStack

import numpy as np

B, N, D = 4, 4096, 128
P = 128
N_CORES = 8

_PROG = None


def _build_program(n=N, i_core=N // 2, d=D, w=512, jt_per_dma=4):
    from concourse import mybir, tile, bacc

    f32 = mybir.dt.float32
    f32r = mybir.dt.float32r
    bf16 = mybir.dt.bfloat16
    n_jt = n // P              # 32 j-tiles of 128
    n_rounds = i_core // w     # 4 i-rounds of 512
    n_chunks = n_jt // jt_per_dma
    xt_chunk = 8               # j-tiles per xT DMA (spread across rings)

    nc = bacc.Bacc(
        "TRN2",
        target_bir_lowering=False,
        debug=False,
        enable_asserts=True,
        num_devices=N_CORES,
    )
    # host-packed: adj_p[p, u, q, n] = adj[b, u*128+p, i0 + q*512 + n]
    adj_d = nc.dram_tensor("adj_p", [P, n_jt, n_rounds, w], bf16, kind="ExternalInput")
    # host-packed transpose: xT_p[d, t, j] = x[b, t*128+j, d]
    xt_d = nc.dram_tensor("xT_p", [P, n_jt, d], bf16, kind="ExternalInput")
    u_d = nc.dram_tensor("U", [d, d], bf16, kind="ExternalInput")
    ones_d = nc.dram_tensor("ones_c", [P, 1], bf16, kind="ExternalInput")
    # out_sp[q, e, n] = out[b, i0 + q*512 + n, e]  (host transposes back)
    out_d = nc.dram_tensor("out_sp", [n_rounds, d, w], bf16, kind="ExternalOutput")

    with tile.TileContext(nc, trace_sim=False) as tc, ExitStack() as ctx:
        const_pool = ctx.enter_context(tc.tile_pool(name="const", bufs=1))
        y_pool = ctx.enter_context(tc.tile_pool(name="y", bufs=1))
        adj_pool = ctx.enter_context(tc.tile_pool(name="adj", bufs=4))
        scale_pool = ctx.enter_context(tc.tile_pool(name="scale", bufs=4))
        out_pool = ctx.enter_context(tc.tile_pool(name="out", bufs=8))
        small_pool = ctx.enter_context(tc.tile_pool(name="small", bufs=4))
        ps_agg = ctx.enter_context(tc.tile_pool(name="ps_agg", bufs=1, space="PSUM"))
        # 4 banks shared in time: y-precompute tiles first, then one deg
        # accumulator per i-round (separate banks - PSUM accumulation-group
        # zero regions are bank-wide).
        ps_aux = ctx.enter_context(tc.tile_pool(name="ps_aux", bufs=1, space="PSUM"))

        u_sb = const_pool.tile([P, d], bf16)
        nc.sync.dma_start(u_sb[:], u_d[:])
        ones = const_pool.tile([P, 1], bf16)
        nc.sync.dma_start(ones[:], ones_d[:])
        xt_sb = const_pool.tile([P, n_jt, d], bf16)
        for xc in range(n_jt // xt_chunk):
            nc.sync.dma_start(
                xt_sb[:, xc * xt_chunk : (xc + 1) * xt_chunk, :],
                xt_d[:, xc * xt_chunk : (xc + 1) * xt_chunk, :],
            )

        # Phase 0: y = x @ U, quantized to bf16, laid out [j_in_tile, t, e].
        y_sb = y_pool.tile([P, n_jt, d], bf16)
        for t in range(n_jt):
            y_ps = ps_aux.tile([P, d], f32, tag=f"d{t % 4}", name=f"y{t}")
            nc.tensor.matmul(y_ps[:], xt_sb[:, t, :], u_sb[:], start=True, stop=True)
            nc.vector.tensor_copy(y_sb[:, t, :], y_ps[:])

        # Phase 1: stream adjacency once, j-outer. agg round q accumulates
        # into its own PSUM bank; all deg rounds share one bank at
        # partitions 0/32/64/96.
        agg_ps = [ps_agg.tile([P, w], f32, tag=f"agg{q}", name=f"agg{q}")
                  for q in range(n_rounds)]
        deg_ps = [ps_aux.tile([1, w], f32, tag=f"d{q}", name=f"deg{q}")
                  for q in range(n_rounds)]
        for c in range(n_chunks):
            adj_sb = adj_pool.tile([P, jt_per_dma, n_rounds, w], bf16, tag="adj")
            nc.sync.dma_start(
                adj_sb[:], adj_d[:, c * jt_per_dma : (c + 1) * jt_per_dma, :, :]
            )
            first, last = c == 0, c == n_chunks - 1

            def agg_mms():
                if last:
                    order = [(u, q) for q in range(n_rounds)
                             for u in range(jt_per_dma)]
                else:
                    order = [(u, q) for u in range(jt_per_dma)
                             for q in range(n_rounds)]
                for u, q in order:
                    t = c * jt_per_dma + u
                    nc.tensor.matmul(
                        agg_ps[q][:],
                        y_sb[:, t, :],
                        adj_sb[:, u, q, :],
                        start=(first and u == 0),
                        stop=(last and u == jt_per_dma - 1),
                    )

            def deg_mms():
                for u in range(jt_per_dma):
                    for q in range(n_rounds):
                        nc.tensor.matmul(
                            deg_ps[q][:],
                            ones[:],
                            adj_sb[:, u, q, :],
                            start=(first and u == 0),
                            stop=(last and u == jt_per_dma - 1),
                        )

            # Last chunk: deg first so the 1/deg tail overlaps final aggs.
            if last:
                deg_mms()
                agg_mms()
            else:
                agg_mms()
                deg_mms()

        for q in range(n_rounds):
            recip = small_pool.tile([1, w], f32, tag="recip")
            nc.vector.reciprocal_approx_fast(recip[:], deg_ps[q][:])
            rb = scale_pool.tile([P, w], f32, tag="rb")
            nc.gpsimd.partition_broadcast(rb[:], recip[:])
            relu_sb = out_pool.tile([P, w], f32, tag="relu")
            nc.scalar.activation(
                relu_sb[:], agg_ps[q][:], mybir.ActivationFunctionType.Relu
            )
            out_sb = out_pool.tile([P, w], bf16, tag="osb")
            nc.vector.tensor_mul(out_sb[:], relu_sb[:], rb[:])
            (nc.scalar if q % 2 else nc.gpsimd).dma_start(out_d[q, :, :], out_sb[:])

    nc.compile()
    return nc


def _get_program():
    global _PROG
    if _PROG is None:
        _PROG = _build_program()
    return _PROG


def _shard_inputs(x, adj_mat, U):
    import ml_dtypes

    bf16 = ml_dtypes.bfloat16
    i_core = N // 2
    n_jt = N // P
    n_rounds = i_core // 512
    ones_c = np.ones((P, 1), dtype=bf16)
    u_f = np.ascontiguousarray(U.astype(bf16))
    adj_bf = adj_mat.astype(bf16)  # exact: values are 0/1
    in_maps = []
    for c in range(N_CORES):
        b, half = c // 2, c % 2
        i0 = half * i_core
        # [N, i_core] -> [u, p, q, n] -> [p, u, q, n]
        a = adj_bf[b, :, i0 : i0 + i_core].reshape(n_jt, P, n_rounds, 512)
        a = np.ascontiguousarray(a.transpose(1, 0, 2, 3))
        xt = np.ascontiguousarray(
            x[b].reshape(n_jt, P, D).transpose(2, 0, 1)
        ).astype(bf16)
        in_maps.append(
            {"adj_p": a, "xT_p": xt, "U": u_f, "ones_c": ones_c}
        )
    return in_maps


def _run(x, adj_mat, U, trace=False):
    from concourse.bass_utils import run_bass_kernel_spmd

    nc = _get_program()
    in_maps = _shard_inputs(x, adj_mat, U)
    res = run_bass_kernel_spmd(
        nc, in_maps, core_ids=list(range(N_CORES)), trace=trace
    )
    i_core = N // 2
    out = np.empty((B, N, D), dtype=np.float32)
    for c in range(N_CORES):
        b, half = c // 2, c % 2
        i0 = half * i_core
        osp = res.results[c]["out_sp"].astype(np.float32)  # [q, e, n]
        out[b, i0 : i0 + i_core, :] = osp.transpose(0, 2, 1).reshape(i_core, D)
    return out, res


def kernel(x, adj_mat, U):
    out, _ = _run(
        np.asarray(x, dtype=np.float32),
        np.asarray(adj_mat, dtype=np.float32),
        np.asarray(U, dtype=np.float32),
    )
    return out


# revision 17
# speedup vs baseline: 1.4869x; 1.0459x over previous
"""GNN message-passing ConvNet layer on 8 TRN2 NeuronCores (Bass/Tile).

Computes, for x [B=4, N=4096, D=128], adj_mat [B, N, N] (0/1 floats),
U [D, D]:
    deg[b, i] = sum_j adj_mat[b, j, i]
    agg[b, i, :] = sum_j adj[b, j, i] * x[b, j, :]
    out = relu((agg @ U) / deg[..., None])

Sharding: core c handles batch c//2 and destination-node half c%2 — no
collectives; each core reads its own adjacency column slice once.

V3 kernel (per core, memory-bound):
  - Associativity: (A^T x) U == A^T (x U). Precompute y = x @ U once on
    the PE (bf16), then the adjacency pass emits the final pre-relu
    output directly (no U-matmul tail).
  - adj is 0/1: cast to fp8e4 on host (exact) -> 8.4 MiB HBM traffic
    per core (4x less than fp32). Host-packs to SBUF tile order
    [p, u, q, n]; DMAs are contiguous per partition.
  - agg matmuls mix dtypes: bf16 y stationary x fp8 adj moving (legal:
    only fp32 operands must match).
  - deg is computed OFF the tensor engine (it used to double PE time):
    a DVE partition-halving tree (fp8+fp8->fp16 adds, exact integers)
    reduces each chunk 128->64->32 partitions, u-merged and accumulated
    into acc[32, q, n]; four final 32-contraction matmuls produce
    deg[1, n] per round.
  - j-outer loop: stationary y_t serves all 4 i-rounds (amortizes
    LDWEIGHTS); input DMAs issue on one queue in priority order (xT
    first) so the y-precompute isn't stuck behind bulk adjacency.
  - Last chunk orders agg matmuls q-outer so each round's
    relu+scale+store overlaps the remaining rounds' matmuls.
"""

import os
import sys

for _p in ("/opt/trn_rl_repo",):
    if _p not in sys.path and os.path.isdir(_p):
        sys.path.insert(0, _p)

from contextlib import ExitStack

import numpy as np

B, N, D = 4, 4096, 128
P = 128
N_CORES = 8

_PROG = None


def _build_program(n=N, i_core=N // 2, d=D, w=512, jt_per_dma=4):
    from concourse import mybir, tile, bacc

    f32 = mybir.dt.float32
    f16 = mybir.dt.float16
    bf16 = mybir.dt.bfloat16
    fp8 = mybir.dt.float8e4
    n_jt = n // P              # 32 j-tiles of 128
    n_rounds = i_core // w     # 4 i-rounds of 512
    n_chunks = n_jt // jt_per_dma
    xt_chunk = 8

    nc = bacc.Bacc(
        "TRN2",
        target_bir_lowering=False,
        debug=False,
        enable_asserts=True,
        num_devices=N_CORES,
    )
    # host-packed: adj_p[p, u, q, n] = adj[b, u*128+p, i0 + q*512 + n]
    adj_d = nc.dram_tensor("adj_p", [P, n_jt, n_rounds, w], fp8, kind="ExternalInput")
    # host-packed transpose: xT_p[d, t, j] = x[b, t*128+j, d]
    xt_d = nc.dram_tensor("xT_p", [P, n_jt, d], bf16, kind="ExternalInput")
    u_d = nc.dram_tensor("U", [d, d], bf16, kind="ExternalInput")
    ones_d = nc.dram_tensor("ones_c", [P, 1], bf16, kind="ExternalInput")
    # out_sp[q, e, n] = out[b, i0 + q*512 + n, e]  (host transposes back)
    out_d = nc.dram_tensor("out_sp", [n_rounds, d, w], bf16, kind="ExternalOutput")

    with tile.TileContext(nc, trace_sim=False) as tc, ExitStack() as ctx:
        const_pool = ctx.enter_context(tc.tile_pool(name="const", bufs=1))
        y_pool = ctx.enter_context(tc.tile_pool(name="y", bufs=1))
        adj_pool = ctx.enter_context(tc.tile_pool(name="adj", bufs=4))
        tree_pool = ctx.enter_context(tc.tile_pool(name="tree", bufs=2))
        acc_pool = ctx.enter_context(tc.tile_pool(name="acc", bufs=2))
        scale_pool = ctx.enter_context(tc.tile_pool(name="scale", bufs=4))
        out_pool = ctx.enter_context(tc.tile_pool(name="out", bufs=8))
        small_pool = ctx.enter_context(tc.tile_pool(name="small", bufs=4))
        ps_agg = ctx.enter_context(tc.tile_pool(name="ps_agg", bufs=1, space="PSUM"))
        # 4 banks shared in time: y-precompute tiles first, then one deg
        # accumulator per i-round.
        ps_aux = ctx.enter_context(tc.tile_pool(name="ps_aux", bufs=1, space="PSUM"))

        u_sb = const_pool.tile([P, d], bf16)
        nc.sync.dma_start(u_sb[:], u_d[:])
        ones = const_pool.tile([P, 1], bf16)
        nc.sync.dma_start(ones[:], ones_d[:])
        xt_sb = const_pool.tile([P, n_jt, d], bf16)
        for xc in range(n_jt // xt_chunk):
            nc.sync.dma_start(
                xt_sb[:, xc * xt_chunk : (xc + 1) * xt_chunk, :],
                xt_d[:, xc * xt_chunk : (xc + 1) * xt_chunk, :],
            )

        # Phase 0: y = x @ U (bf16), laid out [j_in_tile, t, e].
        y_sb = y_pool.tile([P, n_jt, d], bf16)
        for t in range(n_jt):
            y_ps = ps_aux.tile([P, d], f32, tag=f"d{t % 4}", name=f"y{t}")
            nc.tensor.matmul(y_ps[:], xt_sb[:, t, :], u_sb[:], start=True, stop=True)
            nc.scalar.activation(
                y_sb[:, t, :], y_ps[:], mybir.ActivationFunctionType.Copy
            )

        # Phase 1: stream adjacency once. PE does agg only; DVE does the
        # deg partition tree.
        agg_ps = [ps_agg.tile([P, w], f32, tag=f"agg{q}", name=f"agg{q}")
                  for q in range(n_rounds)]
        acc_prev = None
        for c in range(n_chunks):
            adj_sb = adj_pool.tile([P, jt_per_dma, n_rounds, w], fp8, tag="adj")
            nc.sync.dma_start(
                adj_sb[:], adj_d[:, c * jt_per_dma : (c + 1) * jt_per_dma, :, :]
            )
            first, last = c == 0, c == n_chunks - 1

            if last:
                order = [(u, q) for q in range(n_rounds)
                         for u in range(jt_per_dma)]
            else:
                order = [(u, q) for u in range(jt_per_dma)
                         for q in range(n_rounds)]
            for u, q in order:
                t = c * jt_per_dma + u
                nc.tensor.matmul(
                    agg_ps[q][:],
                    y_sb[:, t, :],
                    adj_sb[:, u, q, :],
                    start=(first and u == 0),
                    stop=(last and u == jt_per_dma - 1),
                )

            # deg partials on DVE along the FREE axis only (tensor_tensor
            # requires equal base partitions): merge the chunk's 4 j-tiles
            # into [128, q, n] per-partition counts, accumulate across
            # chunks. Counts stay tiny (<=32) so fp16 is exact.
            t1 = tree_pool.tile([P, n_rounds, w], f16, tag="t1")
            nc.vector.tensor_add(t1[:], adj_sb[:, 0, :, :], adj_sb[:, 1, :, :])
            t2 = tree_pool.tile([P, n_rounds, w], f16, tag="t2")
            nc.vector.tensor_add(t2[:], adj_sb[:, 2, :, :], adj_sb[:, 3, :, :])
            acc = acc_pool.tile([P, n_rounds, w], f16, tag="acc")
            if first:
                nc.vector.tensor_add(acc[:], t1[:], t2[:])
            else:
                t3 = tree_pool.tile([P, n_rounds, w], f16, tag="t3")
                nc.vector.tensor_add(t3[:], t1[:], t2[:])
                nc.vector.tensor_add(acc[:], acc_prev[:], t3[:])
            acc_prev = acc

        # deg[q] = ones^T @ acc[:, q, :] (128-contraction), then tails.
        for q in range(n_rounds):
            deg_ps = ps_aux.tile([1, w], f32, tag=f"d{q}", name=f"deg{q}")
            nc.tensor.matmul(
                deg_ps[:], ones[:], acc_prev[:, q, :], start=True, stop=True
            )
            recip = small_pool.tile([1, w], f32, tag="recip")
            nc.vector.reciprocal_approx_fast(recip[:], deg_ps[:])
            rb = scale_pool.tile([P, w], f32, tag="rb")
            nc.gpsimd.partition_broadcast(rb[:], recip[:])
            relu_sb = out_pool.tile([P, w], f32, tag="relu")
            nc.scalar.activation(
                relu_sb[:], agg_ps[q][:], mybir.ActivationFunctionType.Relu
            )
            out_sb = out_pool.tile([P, w], bf16, tag="osb")
            nc.vector.tensor_mul(out_sb[:], relu_sb[:], rb[:])
            (nc.scalar if q % 2 else nc.gpsimd).dma_start(out_d[q, :, :], out_sb[:])

    nc.compile()
    return nc


def _get_program():
    global _PROG
    if _PROG is None:
        _PROG = _build_program()
    return _PROG


def _shard_inputs(x, adj_mat, U):
    import ml_dtypes

    bf16 = ml_dtypes.bfloat16
    fp8 = ml_dtypes.float8_e4m3
    i_core = N // 2
    n_jt = N // P
    n_rounds = i_core // 512
    ones_c = np.ones((P, 1), dtype=bf16)
    u_f = np.ascontiguousarray(U.astype(bf16))
    adj_f8 = adj_mat.astype(fp8)  # exact: values are 0/1
    in_maps = []
    for c in range(N_CORES):
        b, half = c // 2, c % 2
        i0 = half * i_core
        # [N, i_core] -> [u, p, q, n] -> [p, u, q, n]
        a = adj_f8[b, :, i0 : i0 + i_core].reshape(n_jt, P, n_rounds, 512)
        a = np.ascontiguousarray(a.transpose(1, 0, 2, 3))
        xt = np.ascontiguousarray(
            x[b].reshape(n_jt, P, D).transpose(2, 0, 1)
        ).astype(bf16)
        in_maps.append(
            {"adj_p": a, "xT_p": xt, "U": u_f, "ones_c": ones_c}
        )
    return in_maps


def _run(x, adj_mat, U, trace=False):
    from concourse.bass_utils import run_bass_kernel_spmd

    nc = _get_program()
    in_maps = _shard_inputs(x, adj_mat, U)
    res = run_bass_kernel_spmd(
        nc, in_maps, core_ids=list(range(N_CORES)), trace=trace
    )
    i_core = N // 2
    out = np.empty((B, N, D), dtype=np.float32)
    for c in range(N_CORES):
        b, half = c // 2, c % 2
        i0 = half * i_core
        osp = res.results[c]["out_sp"].astype(np.float32)  # [q, e, n]
        out[b, i0 : i0 + i_core, :] = osp.transpose(0, 2, 1).reshape(i_core, D)
    return out, res


def kernel(x, adj_mat, U):
    out, _ = _run(
        np.asarray(x, dtype=np.float32),
        np.asarray(adj_mat, dtype=np.float32),
        np.asarray(U, dtype=np.float32),
    )
    return out


# revision 20
# speedup vs baseline: 1.8234x; 1.2263x over previous
"""GNN message-passing ConvNet layer on 8 TRN2 NeuronCores (Bass/Tile).

Computes, for x [B=4, N=4096, D=128], adj_mat [B, N, N] (0/1 floats),
U [D, D]:
    deg[b, i] = sum_j adj_mat[b, j, i]
    agg[b, i, :] = sum_j adj[b, j, i] * x[b, j, :]
    out = relu((agg @ U) / deg[..., None])

Sharding: core c handles batch c//2 and destination-node half c%2 — no
collectives; each core reads its own adjacency column slice once.

V3 kernel (per core, memory-bound):
  - Associativity: (A^T x) U == A^T (x U). Precompute y = x @ U once on
    the PE (bf16), then the adjacency pass emits the final pre-relu
    output directly (no U-matmul tail).
  - adj is 0/1: cast to fp8e4 on host (exact) -> 8.4 MiB HBM traffic
    per core (4x less than fp32). Host-packs to SBUF tile order
    [p, u, q, n]; DMAs are contiguous per partition.
  - agg matmuls mix dtypes: bf16 y stationary x fp8 adj moving (legal:
    only fp32 operands must match).
  - deg is computed OFF the tensor engine (it used to double PE time):
    a DVE partition-halving tree (fp8+fp8->fp16 adds, exact integers)
    reduces each chunk 128->64->32 partitions, u-merged and accumulated
    into acc[32, q, n]; four final 32-contraction matmuls produce
    deg[1, n] per round.
  - j-outer loop: stationary y_t serves all 4 i-rounds (amortizes
    LDWEIGHTS); input DMAs issue on one queue in priority order (xT
    first) so the y-precompute isn't stuck behind bulk adjacency.
  - Last chunk orders agg matmuls q-outer so each round's
    relu+scale+store overlaps the remaining rounds' matmuls.
"""

import os
import sys

for _p in ("/opt/trn_rl_repo",):
    if _p not in sys.path and os.path.isdir(_p):
        sys.path.insert(0, _p)

from contextlib import ExitStack

import numpy as np

B, N, D = 4, 4096, 128
P = 128
N_CORES = 8

_PROG = None


def _build_program(n=N, i_core=N // 2, d=D, w=512, jt_per_dma=4):
    from concourse import mybir, tile, bacc

    f32 = mybir.dt.float32
    f16 = mybir.dt.float16
    bf16 = mybir.dt.bfloat16
    fp8 = mybir.dt.float8e4
    n_jt = n // P              # 32 j-tiles of 128
    n_rounds = i_core // w     # 4 i-rounds of 512
    n_chunks = n_jt // jt_per_dma
    xt_chunk = 8

    nc = bacc.Bacc(
        "TRN2",
        target_bir_lowering=False,
        debug=False,
        enable_asserts=True,
        num_devices=N_CORES,
    )
    # host-packed: adj_p[p, v, q, k, n] = adj[b, v*256+k*128+p, i0+q*512+n]
    n_v = n_jt // 2
    adj_d = nc.dram_tensor(
        "adj_p", [P, n_v, n_rounds, 2, w], fp8, kind="ExternalInput"
    )
    # host-packed transpose: xT_p[d, t, j] = x[b, t*128+j, d]
    xt_d = nc.dram_tensor("xT_p", [P, n_jt, d], bf16, kind="ExternalInput")
    u_d = nc.dram_tensor("U", [d, d], bf16, kind="ExternalInput")
    ones_d = nc.dram_tensor("ones_c", [P, 1], bf16, kind="ExternalInput")
    # out_sp[q, e, n] = out[b, i0 + q*512 + n, e]  (host transposes back)
    out_d = nc.dram_tensor("out_sp", [n_rounds, d, w], bf16, kind="ExternalOutput")

    with tile.TileContext(nc, trace_sim=False) as tc, ExitStack() as ctx:
        const_pool = ctx.enter_context(tc.tile_pool(name="const", bufs=1))
        y_pool = ctx.enter_context(tc.tile_pool(name="y", bufs=1))
        adj_pool = ctx.enter_context(tc.tile_pool(name="adj", bufs=4))
        scale_pool = ctx.enter_context(tc.tile_pool(name="scale", bufs=4))
        out_pool = ctx.enter_context(tc.tile_pool(name="out", bufs=8))
        small_pool = ctx.enter_context(tc.tile_pool(name="small", bufs=4))
        ps_agg = ctx.enter_context(tc.tile_pool(name="ps_agg", bufs=1, space="PSUM"))
        # 4 banks shared in time: y-precompute tiles first, then one deg
        # accumulator per i-round.
        ps_aux = ctx.enter_context(tc.tile_pool(name="ps_aux", bufs=1, space="PSUM"))

        u_sb = const_pool.tile([P, d], bf16)
        nc.sync.dma_start(u_sb[:], u_d[:])
        # fp8 pair-of-ones stationary for the DoubleRow deg matmuls
        # [128, 2, 16] pair-of-ones: dim width 16 keeps the DoubleRow
        # LDWEIGHTS pair-axis step a multiple of 16 bytes (ISA rule).
        ones_f8 = const_pool.tile([P, 2, 16], fp8)
        nc.vector.memset(ones_f8[:], 1.0)
        ones = const_pool.tile([P, 1], bf16)
        nc.sync.dma_start(ones[:], ones_d[:])
        xt_sb = const_pool.tile([P, n_jt, d], bf16)
        for xc in range(n_jt // xt_chunk):
            nc.sync.dma_start(
                xt_sb[:, xc * xt_chunk : (xc + 1) * xt_chunk, :],
                xt_d[:, xc * xt_chunk : (xc + 1) * xt_chunk, :],
            )

        # Phase 0: y = x @ U (bf16), laid out [j_in_tile, t, e].
        y_sb = y_pool.tile([P, n_jt, d], bf16)
        for t in range(n_jt):
            y_ps = ps_aux.tile([P, d], f32, tag=f"d{t % 4}", name=f"y{t}")
            nc.tensor.matmul(y_ps[:], xt_sb[:, t, :], u_sb[:], start=True, stop=True)
            nc.scalar.activation(
                y_sb[:, t, :], y_ps[:], mybir.ActivationFunctionType.Copy
            )

        # Phase 1: stream adjacency once. agg via normal mixed-dtype
        # matmuls (bf16 y x fp8 adj); deg via DoubleRow fp8 matmuls
        # (pair-of-ones stationary, k-paired adjacency moving: 256-row
        # contraction in 512 cycles).
        agg_ps = [ps_agg.tile([P, w], f32, tag=f"agg{q}", name=f"agg{q}")
                  for q in range(n_rounds)]
        deg_ps = [ps_aux.tile([16, w], f32, tag=f"d{q}", name=f"deg{q}")
                  for q in range(n_rounds)]
        v_per_dma = jt_per_dma // 2
        for c in range(n_chunks):
            adj_sb = adj_pool.tile(
                [P, v_per_dma, n_rounds, 2, w], fp8, tag="adj"
            )
            nc.sync.dma_start(
                adj_sb[:], adj_d[:, c * v_per_dma : (c + 1) * v_per_dma, :, :, :]
            )
            first, last = c == 0, c == n_chunks - 1

            def deg_mms():
                for vi in range(v_per_dma):
                    for q in range(n_rounds):
                        nc.tensor.matmul(
                            deg_ps[q][:],
                            ones_f8[:],
                            adj_sb[:, vi, q, :, :],
                            start=(first and vi == 0),
                            stop=(last and vi == v_per_dma - 1),
                            perf_mode=mybir.MatmulPerfMode.DoubleRow,
                        )

            def agg_mms():
                if last:
                    order = [(vi, k, q) for q in range(n_rounds)
                             for vi in range(v_per_dma) for k in range(2)]
                else:
                    order = [(vi, k, q) for vi in range(v_per_dma)
                             for k in range(2) for q in range(n_rounds)]
                for vi, k, q in order:
                    t = 2 * (c * v_per_dma + vi) + k
                    nc.tensor.matmul(
                        agg_ps[q][:],
                        y_sb[:, t, :],
                        adj_sb[:, vi, q, k, :],
                        start=(first and vi == 0 and k == 0),
                        stop=(last and vi == v_per_dma - 1 and k == 1),
                    )

            if last:
                deg_mms()
                agg_mms()
            else:
                agg_mms()
                deg_mms()

        for q in range(n_rounds):
            recip = small_pool.tile([1, w], f32, tag="recip")
            nc.vector.reciprocal_approx_fast(recip[:], deg_ps[q][0:1, :])
            rb = scale_pool.tile([P, w], f32, tag="rb")
            nc.gpsimd.partition_broadcast(rb[:], recip[:])
            relu_sb = out_pool.tile([P, w], f32, tag="relu")
            nc.scalar.activation(
                relu_sb[:], agg_ps[q][:], mybir.ActivationFunctionType.Relu
            )
            out_sb = out_pool.tile([P, w], bf16, tag="osb")
            nc.vector.tensor_mul(out_sb[:], relu_sb[:], rb[:])
            (nc.scalar if q % 2 else nc.gpsimd).dma_start(out_d[q, :, :], out_sb[:])

    nc.compile()
    return nc


def _get_program():
    global _PROG
    if _PROG is None:
        _PROG = _build_program()
    return _PROG


def _shard_inputs(x, adj_mat, U):
    import ml_dtypes

    bf16 = ml_dtypes.bfloat16
    fp8 = ml_dtypes.float8_e4m3
    i_core = N // 2
    n_jt = N // P
    n_rounds = i_core // 512
    ones_c = np.ones((P, 1), dtype=bf16)
    u_f = np.ascontiguousarray(U.astype(bf16))
    adj_f8 = adj_mat.astype(fp8)  # exact: values are 0/1
    in_maps = []
    for c in range(N_CORES):
        b, half = c // 2, c % 2
        i0 = half * i_core
        # [N, i_core] -> [v, k, p, q, n] -> [p, v, q, k, n]
        a = adj_f8[b, :, i0 : i0 + i_core].reshape(n_jt // 2, 2, P, n_rounds, 512)
        a = np.ascontiguousarray(a.transpose(2, 0, 3, 1, 4))
        xt = np.ascontiguousarray(
            x[b].reshape(n_jt, P, D).transpose(2, 0, 1)
        ).astype(bf16)
        in_maps.append(
            {"adj_p": a, "xT_p": xt, "U": u_f, "ones_c": ones_c}
        )
    return in_maps


def _run(x, adj_mat, U, trace=False):
    from concourse.bass_utils import run_bass_kernel_spmd

    nc = _get_program()
    in_maps = _shard_inputs(x, adj_mat, U)
    res = run_bass_kernel_spmd(
        nc, in_maps, core_ids=list(range(N_CORES)), trace=trace
    )
    i_core = N // 2
    out = np.empty((B, N, D), dtype=np.float32)
    for c in range(N_CORES):
        b, half = c // 2, c % 2
        i0 = half * i_core
        osp = res.results[c]["out_sp"].astype(np.float32)  # [q, e, n]
        out[b, i0 : i0 + i_core, :] = osp.transpose(0, 2, 1).reshape(i_core, D)
    return out, res


def kernel(x, adj_mat, U):
    out, _ = _run(
        np.asarray(x, dtype=np.float32),
        np.asarray(adj_mat, dtype=np.float32),
        np.asarray(U, dtype=np.float32),
    )
    return out
